# revision 11
# baseline (speedup 1.0000x reference)
"""Causal self-attention (GPT-style, B=2, T=4096, C=768, 12 heads) on 8 TRN2
NeuronCores.

Sharding: core c handles batch b = c//4 and heads [3g, 3g+1, 3g+2] with
g = c%4 (data parallel on B x tensor parallel on heads).  Each core computes
its heads' attention output projected through its slice of w_proj; the host
sums the 4 partial [T, C] outputs per batch and adds b_proj.

Device-side formulation (all matmuls bf16, fp32 accumulate):
  - host passes x[b].T so QKV projections contract C on partitions:
      qT/kT  = W.T @ x.T        -> [head_dim(=partitions), T]
      V'     = x @ [Wv|0] + ones-col -> [T(=partitions), 3*65]  (col 64 of
               each 65-block is constant 1 -> PV also yields softmax denoms)
  - scores computed transposed, S^T[k, q] via lhsT=kT, rhs=qT; two heads per
    512-cycle slot via PE row-tiling (K=64 each, concurrent row groups).
  - softmax without max-subtraction (scores are O(5), exp safe in fp32):
      P^T = exp(0.125 * S^T) on ScalarE, PSUM->SBUF bf16, one activate per
    [128, 1024] (both heads / both k-chunks of a slot share it).
  - causal: strictly-masked k-chunks never computed; at boundary tiles the
    score matmul / exp / PV shrink to the valid column subrange and only the
    diagonal [128,128] strip is multiplied by a triangular 0/1 bf16 mask.
  - PV: oT'[65, q] += V'[k,65].T @ P^T[k,q] accumulated over k-chunks; row 64
    is the softmax denominator.  Normalize: approx-reciprocal on DVE, GpSimd
    partition_broadcast across the 64 head-dim partitions, one DVE multiply.
  - output projection (packed K=128): out[t,:] = yTa[:,t].T @ wp[h01-rows] +
    yT2[:,t].T @ wp[h2-rows].

Schedule (the key difference vs the naive per-phase loop): the ScalarE exp
stream is the critical resource (~1.03us per slot, 216 slots/core), so the
whole kernel is driven as ONE flat sequence of attention slots with the
score matmuls emitted two slots ahead (double-buffered PSUM) ACROSS phase
and q-chunk boundaries, keeping the exp cadence unbroken.  All other PE work
(QKV projection, V' projection, output projection) is chopped into ~0.4-0.8us
filler units in a FIFO queue, popped 1-3 per slot under a slack-credit model
so TensorE stays fed without starving the exp stream.  PSUM banks: 4 for the
two in-flight score tiles, 2 for the PV accumulators, 2 shared by the
qkv/proj/V' accumulation chains.
"""

import numpy as np

N_CORES = 8
B = 2
T = 4096
C = 768
NH = 12
HD = 64
HPC = 3            # heads per core
TCH = 512          # t / q chunk
KCH = 128          # k chunk
CPART = 128

_cache = {}


def _ensure_axon_hooks_module():
    """Make `from antenv.axon_hooks import ...` importable even on images
    whose antenv package lacks the module (profiling then degrades to a
    no-op instead of crashing run_bass_kernel_spmd(trace=True))."""
    import sys
    import types
    try:
        import antenv.axon_hooks  # noqa: F401
        return
    except Exception:
        pass
    m = types.ModuleType("antenv.axon_hooks")
    m._hook = None

    def _set(h):
        m._hook = h

    def _get():
        return m._hook

    m.set_axon_ntff_profile_hook = _set
    m.get_axon_ntff_profile_hook = _get
    sys.modules["antenv.axon_hooks"] = m


def build_program(t=T):
    """Build the single-core SPMD bass program (same program on all cores,
    per-core data). Returns the un-finalized Bacc."""
    import concourse.mybir as mybir
    import concourse.tile as tile
    from concourse import bacc
    from concourse.bass import ds, ts

    f32 = mybir.dt.float32
    bf16 = mybir.dt.bfloat16
    AF = mybir.ActivationFunctionType

    nt = t // TCH          # number of t/q chunks
    spk = TCH // KCH       # k-chunks per t-chunk (4)
    cc_n = C // CPART      # 6 contraction chunks

    nc = bacc.Bacc("TRN2", target_bir_lowering=False)

    # packed bf16 constants: [wq01 768 | wk01 768 | wqk2 768 | wv 1170 |
    #  wpA 768 | wpB 768 (rows 0:64) | tri 128 | misc 384 (row0: bv1+ones128)]
    PK_W = 6 * 128 * 3 + 6 * 195 + C + C + 128 + 384
    xT = nc.dram_tensor("xT", [128, (t // TCH) * (C // CPART) * TCH], bf16,
                        kind="ExternalInput")
    wpk_d = nc.dram_tensor("wpk", [128, PK_W], bf16, kind="ExternalInput")
    bpk_d = nc.dram_tensor("bpk", [128, 3], f32, kind="ExternalInput")
    out_d = nc.dram_tensor("out", [t, C], bf16, kind="ExternalOutput")

    with tile.TileContext(nc) as tc_:
        with (
            tc_.tile_pool(name="consts", bufs=1) as consts,
            tc_.tile_pool(name="big", bufs=1) as big,
            tc_.tile_pool(name="xin", bufs=3) as xin,
            tc_.tile_pool(name="ptp", bufs=8) as ptp,
            tc_.tile_pool(name="wkp", bufs=4) as wkp,
            tc_.tile_pool(name="sps", bufs=2, space="PSUM") as sps,
            tc_.tile_pool(name="otp", bufs=2, space="PSUM") as otp,
            tc_.tile_pool(name="shp", bufs=2, space="PSUM") as shp,
        ):
            # ---- packed weight / bias loads ----
            wpk = consts.tile([128, PK_W], bf16)
            bpk = consts.tile([128, 3], f32)

            def seg(off, w):
                ap = wpk[:, off:off + w]
                return ap, off + w

            _o = 0
            wq01_f, _o = seg(_o, 6 * 128)
            wk01_f, _o = seg(_o, 6 * 128)
            wqk2_f, _o = seg(_o, 6 * 128)
            wv_f, _o = seg(_o, 6 * 195)
            wpA_sb, _o = seg(_o, C)
            wpB_full, _o = seg(_o, C)
            tri_sb, _o = seg(_o, 128)
            misc_f, _o = seg(_o, 384)
            wq01_sb = wq01_f.rearrange("p (c m) -> p c m", c=cc_n)
            wk01_sb = wk01_f.rearrange("p (c m) -> p c m", c=cc_n)
            wqk2_sb = wqk2_f.rearrange("p (c m) -> p c m", c=cc_n)
            wv_sb = wv_f.rearrange("p (c m) -> p c m", c=cc_n)
            wpB_lo = wpB_full[0:64, :]
            del misc_f  # reserved pack space, currently unused
            bq01_sb = bpk[:, 0:1]
            bk01_sb = bpk[:, 1:2]
            bqk2_sb = bpk[:, 2:3]

            # ---- persistent activations ----
            Q01 = big.tile([128, t], bf16)   # rows 0-63 qT_h0, 64-127 qT_h1
            K01 = big.tile([128, t], bf16)
            Q2 = big.tile([128, t], bf16)    # qT_h2 duplicated on both halves
            K2 = big.tile([128, t], bf16)
            Vp = big.tile([128, t // KCH, 195], bf16)
            yTa = big.tile([128, t], bf16)   # normalized h0 (0:64) | h1
            yT2 = big.tile([64, t], bf16)    # h2

            xT_r = xT[:, :].rearrange("p (nt c m) -> p nt c m", nt=nt,
                                      c=cc_n)

            # ---- filler units: (tag, cost_ns, fn) FIFO ----
            from collections import deque
            fq = deque()
            xtb_by_tc = {}
            pending_qkv = {}

            def fq_pop():
                tag, cost, fn = fq.popleft()
                if tag[0] == "qkv":
                    pending_qkv[tag[1]] -= 1
                fn()
                return cost

            def unit_xdma(tci):
                def fn():
                    xtb = xin.tile([128, cc_n, TCH], bf16, tag="xtb",
                                   name="xtb")
                    if tci == 0:
                        # two pieces so the first qk matmuls start early
                        nc.gpsimd.dma_start(xtb[:, 0:3, :],
                                            xT_r[:, tci, 0:3, :])
                        nc.gpsimd.dma_start(xtb[:, 3:6, :],
                                            xT_r[:, tci, 3:6, :])
                    else:
                        nc.gpsimd.dma_start(xtb[:], xT_r[:, tci, :, :])
                    xtb_by_tc[tci] = xtb
                return ("qkv", tci), 30, fn

            def qk_units(tci, wsb, bsb, dst):
                """Six accumulating matmuls + bias, as 3+1 units."""
                st8 = {}

                def mk(cc0):
                    def fn():
                        if cc0 == 0:
                            st8["ps"] = shp.tile([128, TCH], f32, tag="sh",
                                                 name="qkps")
                        qkps = st8["ps"]
                        xtb = xtb_by_tc[tci]
                        for cc in (cc0, cc0 + 1):
                            nc.tensor.matmul(
                                qkps[:], wsb[:, cc, :], xtb[:, cc, :],
                                start=(cc == 0), stop=(cc == cc_n - 1))
                    return fn

                def bias_fn():
                    qkps = st8.pop("ps")
                    if dst is None:
                        # packed [qT_h2; kT_h2]: bias-add the aligned halves
                        # into Q2/K2, then DMA-duplicate across halves.
                        nc.vector.tensor_scalar_add(
                            Q2[0:64, ts(tci, TCH)], qkps[0:64, :],
                            bsb[0:64, :])
                        nc.vector.tensor_scalar_add(
                            K2[64:128, ts(tci, TCH)], qkps[64:128, :],
                            bsb[64:128, :])
                        nc.sync.dma_start(Q2[64:128, ts(tci, TCH)],
                                          Q2[0:64, ts(tci, TCH)])
                        nc.sync.dma_start(K2[0:64, ts(tci, TCH)],
                                          K2[64:128, ts(tci, TCH)])
                    else:
                        nc.vector.tensor_scalar_add(
                            dst[:, ts(tci, TCH)], qkps[:], bsb[:])

                return ([(("qkv", tci), 490, mk(cc0)) for cc0 in (0, 2, 4)]
                        + [(("qkv", tci), 60, bias_fn)])

            def v_units(tci, st):
                """V' projection for one 128-token slice: 3+3 matmuls+copy."""
                tt = tci * spk + st
                st8 = {}

                def fn_a():
                    st8["ps"] = shp.tile([128, 195], f32, tag="sh",
                                         name="vps")
                    xtb = xtb_by_tc[tci]
                    for cc in (0, 1, 2):
                        nc.tensor.matmul(
                            st8["ps"][:], xtb[:, cc, ts(st, 128)],
                            wv_sb[:, cc, :],
                            start=(cc == 0), stop=False)

                def fn_b():
                    vps = st8.pop("ps")
                    xtb = xtb_by_tc[tci]
                    for cc in (3, 4, 5):
                        nc.tensor.matmul(
                            vps[:], xtb[:, cc, ts(st, 128)], wv_sb[:, cc, :],
                            start=False, stop=(cc == cc_n - 1))
                    nc.vector.tensor_copy(Vp[:, tt, :], vps[:])
                    # denominator ones columns (cols 64/129/194 of each row)
                    nc.vector.memset(
                        Vp[:, tt, :].rearrange("p (a b) -> p a b", b=65)[
                            :, :, 64], 1.0)

                return [(("qkv", tci), 420, fn_a), (("qkv", tci), 480, fn_b)]

            def qkv_all_units(tci):
                u = [unit_xdma(tci)]
                u += qk_units(tci, wq01_sb, bq01_sb, Q01)
                u += qk_units(tci, wk01_sb, bk01_sb, K01)
                u += qk_units(tci, wqk2_sb, bqk2_sb, None)
                for st in range(spk):
                    u += v_units(tci, st)
                return u

            def proj_unit(tci, st, tail=False):
                tt = tci * spk + st

                def fn():
                    po1 = shp.tile([128, 512], f32, tag="sh", name="po1")
                    po2 = shp.tile([128, 256], f32, tag="sh", name="po2")
                    for po, cs, cw in ((po1, 0, 512), (po2, 512, 256)):
                        nc.tensor.matmul(po[:], yTa[:, ts(tt, 128)],
                                         wpA_sb[:, ds(cs, cw)],
                                         start=True, stop=False)
                        nc.tensor.matmul(po[:], yT2[:, ts(tt, 128)],
                                         wpB_lo[:, ds(cs, cw)],
                                         start=False, stop=True)
                    pout = xin.tile([128, C], bf16, tag="pout", name="pout")
                    if tail:
                        # ScalarE is idle after the last exp: split the two
                        # casts across engines so the final tiles pipeline
                        nc.scalar.activation(pout[:, 0:512], po1[:], AF.Copy)
                    else:
                        nc.vector.tensor_copy(pout[:, 0:512], po1[:])
                    nc.vector.tensor_copy(pout[:, 512:768], po2[:])
                    nc.sync.dma_start(out_d[ds(tt * 128, 64), :],
                                      pout[0:64, :])
                    nc.sync.dma_start(out_d[ds(tt * 128 + 64, 64), :],
                                      pout[64:128, :])
                return ("proj", tci), 760, fn

            # ---- attention slot machinery ----
            slots = []
            qc_start = []
            for qc in range(nt):
                nkc = (qc + 1) * spk
                qc_start.append(len(slots))
                slots += [("p1", qc, kc) for kc in range(nkc)]
                slots += [("p2", qc, kp) for kp in range(nkc // 2)]
            ns = len(slots)

            s_pend = {}
            oT_cur = {}

            def lo_of(qc, kc):
                m = kc - qc * spk
                return max(0, 128 * m), m

            def emit_scores(i):
                ph, qc, k = slots[i]
                q0 = qc * TCH
                S = sps.tile([128, 1024], f32, tag="S", name="S")
                if ph == "p1":
                    lo, _ = lo_of(qc, k)
                    nc.tensor.matmul(
                        S[:, lo:TCH],
                        K01[0:64, ts(k, KCH)],
                        Q01[0:64, ds(q0 + lo, TCH - lo)],
                        start=True, stop=True, tile_position=(0, 0))
                    nc.tensor.matmul(
                        S[:, TCH + lo:1024],
                        K01[64:128, ts(k, KCH)],
                        Q01[64:128, ds(q0 + lo, TCH - lo)],
                        start=True, stop=True, tile_position=(64, 0))
                else:
                    kc0, kc1 = 2 * k, 2 * k + 1
                    lo0, _ = lo_of(qc, kc0)
                    nc.tensor.matmul(
                        S[:, lo0:TCH],
                        K2[0:64, ts(kc0, KCH)],
                        Q2[0:64, ds(q0 + lo0, TCH - lo0)],
                        start=True, stop=True, tile_position=(0, 0))
                    # kc1 half also starts at lo0 (not lo1) so the merged
                    # strided exp reads fully-initialized PSUM; the extra
                    # [lo0:lo1) columns are causally dead and never read.
                    nc.tensor.matmul(
                        S[:, TCH + lo0:1024],
                        K2[64:128, ts(kc1, KCH)],
                        Q2[64:128, ds(q0 + lo0, TCH - lo0)],
                        start=True, stop=True, tile_position=(64, 0))
                s_pend[i] = S

            def slot_slack(i):
                ph, qc, k = slots[i]
                if ph == "p1":
                    lo, _ = lo_of(qc, k)
                else:
                    lo, _ = lo_of(qc, 2 * k)
                w = TCH - lo
                exp_ns = 2 * w * 0.833 + 270
                pe_ns = w * 0.4167 + 100 + 2 * w * 0.4167 + 120
                return exp_ns - pe_ns

            def normalize(oT, h, qc):
                den = wkp.tile([1, TCH], f32, tag="den", name="den")
                nc.vector.tensor_copy(den[:], oT[64:65, :])
                recip = wkp.tile([1, TCH], f32, tag="recip", name="recip")
                nc.vector.reciprocal_approx_fast(out=recip[:], in_=den[:])
                rb = wkp.tile([64, TCH], f32, tag="rb", name="rb")
                nc.gpsimd.partition_broadcast(rb[:], recip[:])
                if h == 0:
                    nc.vector.tensor_mul(yTa[0:64, ts(qc, TCH)], oT[0:64, :],
                                         rb[:])
                elif h == 2:
                    nc.vector.tensor_mul(yT2[:, ts(qc, TCH)], oT[0:64, :],
                                         rb[:])
                else:
                    y1t = wkp.tile([64, TCH], bf16, tag="y1t", name="y1t")
                    nc.vector.tensor_mul(y1t[:], oT[0:64, :], rb[:])
                    nc.sync.dma_start(yTa[64:128, ts(qc, TCH)], y1t[:])

            credit = [0.0]

            def pump(i):
                """Pop filler units at a deadline-aware pace: the qkv units
                of t-chunk tc must all be emitted by 2 slots before tc's
                first attention slot (emission order = dependency order);
                spread them evenly over the remaining slots.  Extra units
                (proj) drain opportunistically while slack-credit remains."""
                import math
                min_pops = 0
                live = [tc for tc, n in pending_qkv.items() if n > 0]
                if live:
                    tcm = min(live)
                    rem = max(1, qc_start[tcm] - 2 - i)
                    min_pops = math.ceil(sum(pending_qkv[tc] for tc in live)
                                         / rem)
                pops = 0
                while fq and pops < max(min_pops, 4):
                    if pops >= min_pops and (pops >= 2 or credit[0] <= 0):
                        break
                    credit[0] -= fq_pop()
                    pops += 1
                if i + 2 < ns:
                    qn = slots[i + 2][1]
                    if qn != slots[i][1]:
                        # safety net: everything qn needs must be emitted now
                        while fq and pending_qkv.get(qn, 0) > 0:
                            credit[0] -= fq_pop()
                credit[0] += slot_slack(i)
                credit[0] = max(-2000.0, min(3000.0, credit[0]))

            def do_slot(i):
                ph, qc, k = slots[i]
                nkc = (qc + 1) * spk
                npair = nkc // 2
                pump(i)
                S = s_pend.pop(i)
                PT = ptp.tile([128, 1024], bf16, tag="PT", name="PT")
                if ph == "p1":
                    lo, m = lo_of(qc, k)
                    if lo == 0:
                        nc.scalar.activation(PT[:], S[:], AF.Exp, scale=0.125)
                    else:
                        s_v = S[:].rearrange("p (h q) -> p h q", h=2)[
                            :, :, lo:TCH]
                        p_v = PT[:].rearrange("p (h q) -> p h q", h=2)[
                            :, :, lo:TCH]
                        nc.scalar.activation(p_v, s_v, AF.Exp, scale=0.125)
                    if m >= 0:
                        nc.vector.tensor_mul(PT[:, ds(lo, 128)],
                                             PT[:, ds(lo, 128)], tri_sb[:])
                        nc.vector.tensor_mul(PT[:, ds(TCH + lo, 128)],
                                             PT[:, ds(TCH + lo, 128)],
                                             tri_sb[:])
                    if k == 0:
                        oT_cur["o0"] = otp.tile([65, TCH], f32, tag="oT",
                                                name="oT0")
                        oT_cur["o1"] = otp.tile([65, TCH], f32, tag="oT",
                                                name="oT1")
                    oT0, oT1 = oT_cur["o0"], oT_cur["o1"]
                    nc.tensor.matmul(oT0[:, lo:TCH], Vp[:, k, 0:65],
                                     PT[:, lo:TCH],
                                     start=(k == 0), stop=(k == nkc - 1))
                    nc.tensor.matmul(oT1[:, lo:TCH], Vp[:, k, 65:130],
                                     PT[:, TCH + lo:1024],
                                     start=(k == 0), stop=(k == nkc - 1))
                else:
                    kc0, kc1 = 2 * k, 2 * k + 1
                    lo0, m0 = lo_of(qc, kc0)
                    lo1, m1 = lo_of(qc, kc1)
                    if lo0 == 0:
                        nc.scalar.activation(PT[:], S[:], AF.Exp, scale=0.125)
                    else:
                        # one strided activate from lo0 in both halves; the
                        # [lo0:lo1) cols of half 1 are garbage but never read
                        s_v = S[:].rearrange("p (h q) -> p h q", h=2)[
                            :, :, lo0:TCH]
                        p_v = PT[:].rearrange("p (h q) -> p h q", h=2)[
                            :, :, lo0:TCH]
                        nc.scalar.activation(p_v, s_v, AF.Exp, scale=0.125)
                    if m0 >= 0:
                        nc.vector.tensor_mul(PT[:, ds(lo0, 128)],
                                             PT[:, ds(lo0, 128)], tri_sb[:])
                    if m1 >= 0:
                        nc.vector.tensor_mul(PT[:, ds(TCH + lo1, 128)],
                                             PT[:, ds(TCH + lo1, 128)],
                                             tri_sb[:])
                    if k == 0:
                        oT_cur["o2"] = otp.tile([65, TCH], f32, tag="oT",
                                                name="oT2")
                    oT2 = oT_cur["o2"]
                    nc.tensor.matmul(oT2[:, lo0:TCH], Vp[:, kc0, 130:195],
                                     PT[:, lo0:TCH],
                                     start=(k == 0), stop=False)
                    nc.tensor.matmul(oT2[:, lo1:TCH], Vp[:, kc1, 130:195],
                                     PT[:, TCH + lo1:1024],
                                     start=False, stop=(k == npair - 1))
                if i + 2 < ns:
                    emit_scores(i + 2)
                # phase ends: normalize + enqueue downstream work
                if ph == "p1" and k == nkc - 1:
                    normalize(oT_cur.pop("o0"), 0, qc)
                    normalize(oT_cur.pop("o1"), 1, qc)
                if ph == "p2" and k == npair - 1:
                    normalize(oT_cur.pop("o2"), 2, qc)
                    last = qc == nt - 1
                    for st in range(spk):
                        fq.append(proj_unit(qc, st, tail=last))

            # ---- preamble: weights, x(0..1), qkv(0..1), first score tiles.
            # All startup DMAs issue from the (idle) Pool queue: its DGE
            # config time is ~25ns vs ~565ns on SP, so the x/weight
            # transfers start as soon as the framework preamble ends.
            nc.gpsimd.dma_start(wpk[:, 0:768], wpk_d[:, 0:768])
            _, _, fn = unit_xdma(0)
            fn()
            nc.gpsimd.dma_start(bpk[:], bpk_d[:, :])
            # PE warmup on a zeroed SBUF tile while the DMAs land (p-state)
            warm_src = consts.tile([128, 512], bf16)
            nc.vector.memset(warm_src[:], 0.0)
            wps = shp.tile([128, 512], f32, tag="sh", name="warm")
            for _ in range(4):
                nc.tensor.matmul(wps[:], warm_src[:, 0:128], warm_src[:],
                                 start=True, stop=True)
            nc.gpsimd.dma_start(wpk[:, 768:2304], wpk_d[:, 768:2304])
            nc.gpsimd.dma_start(wpk[:, 2304:3474], wpk_d[:, 2304:3474])
            nc.gpsimd.dma_start(wpk[:, 3474:PK_W], wpk_d[:, 3474:PK_W])

            def qk_chain(tci, wsb, bsb, dst, t0, t1):
                """Contiguous 6-matmul projection chain for tokens
                [t0:t1) of t-chunk tci (partial-width: gets the first
                score tile going before the full k chunk is done)."""
                qkps = shp.tile([128, t1 - t0], f32, tag="sh", name="qkps")
                xtb = xtb_by_tc[tci]
                for cc in range(cc_n):
                    nc.tensor.matmul(
                        qkps[:], wsb[:, cc, :], xtb[:, cc, t0:t1],
                        start=(cc == 0), stop=(cc == cc_n - 1))
                nc.vector.tensor_scalar_add(
                    dst[:, tci * TCH + t0:tci * TCH + t1], qkps[:], bsb[:])

            qk_chain(0, wk01_sb, bk01_sb, K01, 0, KCH)
            qk_chain(0, wq01_sb, bq01_sb, Q01, 0, TCH)
            emit_scores(0)
            qk_chain(0, wk01_sb, bk01_sb, K01, KCH, TCH)
            emit_scores(1)
            for _, _, fn in qk_units(0, wqk2_sb, bqk2_sb, None):
                fn()
            for st in range(spk):
                for _, _, fn in v_units(0, st):
                    fn()
            # t-chunk 1 QKV rides the early slots' exp time as PE backlog
            for _, _, fn in qkv_all_units(1):
                fn()

            # ---- main flat slot loop ----
            for i in range(ns):
                ph, qc, k = slots[i]
                if ph == "p1" and k == 0 and qc + 2 < nt:
                    units = qkv_all_units(qc + 2)
                    pending_qkv[qc + 2] = len(units)
                    fq.extend(units)
                do_slot(i)

            # ---- drain remaining fillers (last proj tiles) ----
            while fq:
                fq_pop()

    return nc


def arrange_x(xb):
    """x[b] is [t, C]; device wants xT as [128, nt, cc, TCH] contiguous."""
    import ml_dtypes
    t = xb.shape[0]
    xt = xb.T.reshape(C // CPART, CPART, t // TCH, TCH)
    xt = xt.transpose(1, 2, 0, 3).reshape(CPART, -1)
    return np.ascontiguousarray(xt).astype(ml_dtypes.bfloat16)


def make_tri():
    import ml_dtypes
    p = np.arange(128)[:, None]
    j = np.arange(128)[None, :]
    return (j - p >= 0).astype(ml_dtypes.bfloat16)


def core_inputs(c, x, w_attn, b_attn, w_proj, xT_by_batch, tri):
    import ml_dtypes
    f32 = np.float32
    b = c // 4
    heads = [(c % 4) * HPC + i for i in range(HPC)]
    h0, h1, h2 = heads

    def Wq(h):
        return w_attn[:, h * HD:(h + 1) * HD]

    def Wk(h):
        return w_attn[:, C + h * HD:C + (h + 1) * HD]

    def Wv(h):
        return w_attn[:, 2 * C + h * HD:2 * C + (h + 1) * HD]

    def bq(h):
        return b_attn[h * HD:(h + 1) * HD]

    def bk(h):
        return b_attn[C + h * HD:C + (h + 1) * HD]

    wv195 = np.zeros((C, 195), f32)
    for i, h in enumerate(heads):
        wv195[:, i * 65:i * 65 + 64] = Wv(h)
    bf = ml_dtypes.bfloat16

    def arr(w):
        m = w.shape[1]
        return np.ascontiguousarray(
            w.reshape(C // CPART, CPART, m).transpose(1, 0, 2).reshape(
                CPART, -1)).astype(bf)

    wp192 = np.concatenate([w_proj[h * HD:(h + 1) * HD, :] for h in heads], 0)
    wpB = np.zeros((CPART, C), np.float32)
    wpB[0:64, :] = wp192[128:192, :]
    wpB[64:128, :] = wp192[128:192, :]
    misc = np.zeros((CPART, 384), np.float32)
    wpk = np.concatenate([
        arr(np.concatenate([Wq(h0), Wq(h1)], 1)).astype(np.float32),
        arr(np.concatenate([Wk(h0), Wk(h1)], 1)).astype(np.float32),
        arr(np.concatenate([Wq(h2), Wk(h2)], 1)).astype(np.float32),
        arr(wv195).astype(np.float32),
        wp192[0:128, :], wpB, tri.astype(np.float32), misc,
    ], axis=1).astype(bf)
    bpk = np.stack([
        np.concatenate([bq(h0), bq(h1)]),
        np.concatenate([bk(h0), bk(h1)]),
        np.concatenate([bq(h2), bk(h2)]),
    ], axis=1).astype(np.float32)
    return {
        "xT": xT_by_batch[b],
        "wpk": np.ascontiguousarray(wpk),
        "bpk": np.ascontiguousarray(bpk),
    }


TRACE = False
LAST_EXEC_NS = None
LAST_RESULTS = None


def kernel(x, w_attn, b_attn, w_proj, b_proj):
    global LAST_EXEC_NS, LAST_RESULTS
    _ensure_axon_hooks_module()
    from concourse.bass_utils import run_bass_kernel_spmd

    x = np.asarray(x, np.float32)
    w_attn = np.asarray(w_attn, np.float32)
    b_attn = np.asarray(b_attn, np.float32)
    w_proj = np.asarray(w_proj, np.float32)
    b_proj = np.asarray(b_proj, np.float32)

    if "nc" not in _cache:
        nc = build_program()
        nc.finalize()
        _cache["nc"] = nc
    nc = _cache["nc"]

    import ml_dtypes  # noqa: F401
    xT_by_batch = [arrange_x(x[b]) for b in range(B)]
    tri = make_tri()
    in_maps = [
        core_inputs(c, x, w_attn, b_attn, w_proj, xT_by_batch, tri)
        for c in range(N_CORES)
    ]
    res = run_bass_kernel_spmd(nc, in_maps, core_ids=list(range(N_CORES)),
                               trace=TRACE)
    LAST_EXEC_NS = res.exec_time_ns
    LAST_RESULTS = res
    out = np.zeros((B, T, C), np.float32)
    for c in range(N_CORES):
        out[c // 4] += np.asarray(res.results[c]["out"], np.float32)
    # V-bias commutes out of softmax (weights sum to 1): add bv @ w_proj
    bv_all = b_attn[2 * C:]
    out += (b_proj + bv_all @ w_proj)[None, None, :]
    return out


# revision 13
# speedup vs baseline: 1.0237x; 1.0237x over previous
"""Causal self-attention (GPT-style, B=2, T=4096, C=768, 12 heads) on 8 TRN2
NeuronCores.

Sharding: core c handles batch b = c//4 and heads [3g, 3g+1, 3g+2] with
g = c%4 (data parallel on B x tensor parallel on heads).  Each core computes
its heads' attention output projected through its slice of w_proj; the host
sums the 4 partial [T, C] outputs per batch and adds b_proj.

Device-side formulation (all matmuls bf16, fp32 accumulate):
  - host passes x[b].T so QKV projections contract C on partitions:
      qT/kT  = W.T @ x.T        -> [head_dim(=partitions), T]
      V'     = x @ [Wv|0] + ones-col -> [T(=partitions), 3*65]  (col 64 of
               each 65-block is constant 1 -> PV also yields softmax denoms)
  - scores computed transposed, S^T[k, q] via lhsT=kT, rhs=qT; two heads per
    512-cycle slot via PE row-tiling (K=64 each, concurrent row groups).
  - softmax without max-subtraction (scores are O(5), exp safe in fp32):
      P^T = exp(0.125 * S^T) on ScalarE, PSUM->SBUF bf16, one activate per
    [128, 1024] (both heads / both k-chunks of a slot share it).
  - causal: strictly-masked k-chunks never computed; at boundary tiles the
    score matmul / exp / PV shrink to the valid column subrange and only the
    diagonal [128,128] strip is multiplied by a triangular 0/1 bf16 mask.
  - PV: oT'[65, q] += V'[k,65].T @ P^T[k,q] accumulated over k-chunks; row 64
    is the softmax denominator.  Normalize: approx-reciprocal on DVE, GpSimd
    partition_broadcast across the 64 head-dim partitions, one DVE multiply.
  - output projection (packed K=128): out[t,:] = yTa[:,t].T @ wp[h01-rows] +
    yT2[:,t].T @ wp[h2-rows].

Schedule (the key difference vs the naive per-phase loop): the ScalarE exp
stream is the critical resource (~1.03us per slot, 216 slots/core), so the
whole kernel is driven as ONE flat sequence of attention slots with the
score matmuls emitted two slots ahead (double-buffered PSUM) ACROSS phase
and q-chunk boundaries, keeping the exp cadence unbroken.  All other PE work
(QKV projection, V' projection, output projection) is chopped into ~0.4-0.8us
filler units in a FIFO queue, popped 1-3 per slot under a slack-credit model
so TensorE stays fed without starving the exp stream.  PSUM banks: 4 for the
two in-flight score tiles, 2 for the PV accumulators, 2 shared by the
qkv/proj/V' accumulation chains.
"""

import numpy as np

N_CORES = 8
B = 2
T = 4096
C = 768
NH = 12
HD = 64
HPC = 3            # heads per core
TCH = 512          # t / q chunk
KCH = 128          # k chunk
CPART = 128

_cache = {}


def _ensure_axon_hooks_module():
    """Make `from antenv.axon_hooks import ...` importable even on images
    whose antenv package lacks the module (profiling then degrades to a
    no-op instead of crashing run_bass_kernel_spmd(trace=True))."""
    import sys
    import types
    try:
        import antenv.axon_hooks  # noqa: F401
        return
    except Exception:
        pass
    m = types.ModuleType("antenv.axon_hooks")
    m._hook = None

    def _set(h):
        m._hook = h

    def _get():
        return m._hook

    m.set_axon_ntff_profile_hook = _set
    m.get_axon_ntff_profile_hook = _get
    sys.modules["antenv.axon_hooks"] = m


def build_program(t=T):
    """Build the single-core SPMD bass program (same program on all cores,
    per-core data). Returns the un-finalized Bacc."""
    import math
    from collections import deque

    import concourse.mybir as mybir
    import concourse.tile as tile
    from concourse import bacc
    from concourse.bass import ds, ts

    f32 = mybir.dt.float32
    bf16 = mybir.dt.bfloat16
    AF = mybir.ActivationFunctionType

    nt = t // TCH          # number of t/q chunks
    spk = TCH // KCH       # k-chunks per t-chunk (4)
    cc_n = C // CPART      # 6 contraction chunks

    nc = bacc.Bacc("TRN2", target_bir_lowering=False)

    # packed bf16 constants: [wq01 768 | wk01 768 | wqk2 768 | wv 1170 |
    #  wpA 768 | wpB 768 (rows 0:64) | tri 128 | misc 384]
    PK_W = 6 * 128 * 3 + 6 * 195 + C + C + 128 + 384
    xT = nc.dram_tensor("xT", [128, (t // TCH) * (C // CPART) * TCH], bf16,
                        kind="ExternalInput")
    wpk_d = nc.dram_tensor("wpk", [128, PK_W], bf16, kind="ExternalInput")
    bpk_d = nc.dram_tensor("bpk", [128, 3], f32, kind="ExternalInput")
    out_d = nc.dram_tensor("out", [t, C], bf16, kind="ExternalOutput")

    with tile.TileContext(nc) as tc_:
        with (
            tc_.tile_pool(name="consts", bufs=1) as consts,
            tc_.tile_pool(name="big", bufs=1) as big,
            tc_.tile_pool(name="xin", bufs=3) as xin,
            tc_.tile_pool(name="ptp", bufs=8) as ptp,
            tc_.tile_pool(name="wkp", bufs=4) as wkp,
            tc_.tile_pool(name="sps", bufs=2, space="PSUM") as sps,
            tc_.tile_pool(name="otp", bufs=2, space="PSUM") as otp,
            tc_.tile_pool(name="shp", bufs=2, space="PSUM") as shp,
        ):
            wpk = consts.tile([128, PK_W], bf16)
            bpk = consts.tile([128, 3], f32)

            def seg(off, w):
                ap = wpk[:, off:off + w]
                return ap, off + w

            _o = 0
            wq01_f, _o = seg(_o, 6 * 128)
            wk01_f, _o = seg(_o, 6 * 128)
            wqk2_f, _o = seg(_o, 6 * 128)
            wv_f, _o = seg(_o, 6 * 195)
            wpA_sb, _o = seg(_o, C)
            wpB_full, _o = seg(_o, C)
            tri_sb, _o = seg(_o, 128)
            misc_f, _o = seg(_o, 384)
            wq01_sb = wq01_f.rearrange("p (c m) -> p c m", c=cc_n)
            wk01_sb = wk01_f.rearrange("p (c m) -> p c m", c=cc_n)
            wqk2_sb = wqk2_f.rearrange("p (c m) -> p c m", c=cc_n)
            wv_sb = wv_f.rearrange("p (c m) -> p c m", c=cc_n)
            wpB_lo = wpB_full[0:64, :]
            del misc_f
            bq01_sb = bpk[:, 0:1]
            bk01_sb = bpk[:, 1:2]
            bqk2_sb = bpk[:, 2:3]

            # ---- persistent activations ----
            Q01 = big.tile([128, t], bf16)   # rows 0-63 qT_h0, 64-127 qT_h1
            K01 = big.tile([128, t], bf16)
            Q2 = big.tile([128, t], bf16)    # qT_h2 duplicated on both halves
            K2 = big.tile([128, t], bf16)
            Vp = big.tile([128, t // KCH, 195], bf16)
            yTa = big.tile([128, t], bf16)   # normalized h0 (0:64) | h1
            yT2 = big.tile([64, t], bf16)    # h2

            xT_r = xT[:, :].rearrange("p (nt c m) -> p nt c m", nt=nt,
                                      c=cc_n)

            # ---- slot table ----
            slots = []
            qc_start = []
            for qc in range(nt):
                nkc = (qc + 1) * spk
                qc_start.append(len(slots))
                slots += [("p1", qc, kc) for kc in range(nkc)]
                slots += [("p2", qc, kp) for kp in range(nkc // 2)]
            ns = len(slots)

            def p2_start(qc):
                return qc_start[qc] + (qc + 1) * spk

            # ---- filler units: FIFO of (due_slot, cost, fn) ----
            # Emission order IS dependency order on the in-order engine
            # queues, so every unit carries a due slot: the latest slot
            # index by which it must have been EMITTED.  The pump spreads
            # pops so each deadline is met without bursts.
            fq = deque()
            xtb_by_tc = {}

            def fq_pop():
                due, cost, fn = fq.popleft()
                fn()
                return cost

            def unit_xdma(tci):
                def fn():
                    xtb = xin.tile([128, cc_n, TCH], bf16, tag="xtb",
                                   name="xtb")
                    nc.gpsimd.dma_start(xtb[:], xT_r[:, tci, :, :])
                    xtb_by_tc[tci] = xtb
                return (max(0, qc_start[tci] - 8), 30, fn)

            def qk_units(tci, wsb, bsb, dst, due):
                st8 = {}

                def mk(cc0):
                    def fn():
                        if cc0 == 0:
                            st8["ps"] = shp.tile([128, TCH], f32, tag="sh",
                                                 name="qkps")
                        qkps = st8["ps"]
                        xtb = xtb_by_tc[tci]
                        for cc in (cc0, cc0 + 1):
                            nc.tensor.matmul(
                                qkps[:], wsb[:, cc, :], xtb[:, cc, :],
                                start=(cc == 0), stop=(cc == cc_n - 1))
                    return fn

                def bias_fn():
                    qkps = st8.pop("ps")
                    if dst is None:
                        nc.vector.tensor_scalar_add(
                            Q2[0:64, ts(tci, TCH)], qkps[0:64, :],
                            bsb[0:64, :])
                        nc.vector.tensor_scalar_add(
                            K2[64:128, ts(tci, TCH)], qkps[64:128, :],
                            bsb[64:128, :])
                        nc.sync.dma_start(Q2[64:128, ts(tci, TCH)],
                                          Q2[0:64, ts(tci, TCH)])
                        nc.sync.dma_start(K2[0:64, ts(tci, TCH)],
                                          K2[64:128, ts(tci, TCH)])
                    else:
                        nc.vector.tensor_scalar_add(
                            dst[:, ts(tci, TCH)], qkps[:], bsb[:])

                return ([(due, 490, mk(cc0)) for cc0 in (0, 2, 4)]
                        + [(due, 60, bias_fn)])

            def v_units(tci, st):
                """V' projection for one 128-token slice; due right before
                the diagonal PV slot that reads it."""
                tt = tci * spk + st
                due = qc_start[tci] + tci * spk + st - 1 if tci > 0 else 0
                st8 = {}

                def fn_a():
                    st8["ps"] = shp.tile([128, 195], f32, tag="sh",
                                         name="vps")
                    xtb = xtb_by_tc[tci]
                    for cc in (0, 1, 2):
                        nc.tensor.matmul(
                            st8["ps"][:], xtb[:, cc, ts(st, 128)],
                            wv_sb[:, cc, :],
                            start=(cc == 0), stop=False)

                def fn_b():
                    vps = st8.pop("ps")
                    xtb = xtb_by_tc[tci]
                    for cc in (3, 4, 5):
                        nc.tensor.matmul(
                            vps[:], xtb[:, cc, ts(st, 128)], wv_sb[:, cc, :],
                            start=False, stop=(cc == cc_n - 1))
                    nc.vector.tensor_copy(Vp[:, tt, :], vps[:])
                    nc.vector.memset(
                        Vp[:, tt, :].rearrange("p (a b) -> p a b", b=65)[
                            :, :, 64], 1.0)

                return [(due - 1, 420, fn_a), (due, 480, fn_b)]

            def qkv_all_units(tci):
                u = [unit_xdma(tci)]
                due_qk = qc_start[tci] - 2
                u += qk_units(tci, wq01_sb, bq01_sb, Q01, due_qk)
                u += qk_units(tci, wk01_sb, bk01_sb, K01, due_qk)
                u += qk_units(tci, wqk2_sb, bqk2_sb, None,
                              p2_start(tci) - 2)
                for st in range(spk):
                    u += v_units(tci, st)
                return u

            def proj_unit(tci, st, tail=False):
                tt = tci * spk + st
                due = qc_start[tci + 2] if tci + 2 < nt else ns - 1

                def fn():
                    po1 = shp.tile([128, 512], f32, tag="sh", name="po1")
                    po2 = shp.tile([128, 256], f32, tag="sh", name="po2")
                    for po, cs, cw in ((po1, 0, 512), (po2, 512, 256)):
                        nc.tensor.matmul(po[:], yTa[:, ts(tt, 128)],
                                         wpA_sb[:, ds(cs, cw)],
                                         start=True, stop=False)
                        nc.tensor.matmul(po[:], yT2[:, ts(tt, 128)],
                                         wpB_lo[:, ds(cs, cw)],
                                         start=False, stop=True)
                    pout = xin.tile([128, C], bf16, tag="pout", name="pout")
                    if tail:
                        nc.scalar.activation(pout[:, 0:512], po1[:], AF.Copy)
                    else:
                        nc.vector.tensor_copy(pout[:, 0:512], po1[:])
                    nc.vector.tensor_copy(pout[:, 512:768], po2[:])
                    nc.sync.dma_start(out_d[ds(tt * 128, 64), :],
                                      pout[0:64, :])
                    nc.sync.dma_start(out_d[ds(tt * 128 + 64, 64), :],
                                      pout[64:128, :])
                return (due, 760, fn)

            # ---- attention machinery ----
            s_pend = {}
            oT_cur = {}

            def lo_of(qc, kc):
                m = kc - qc * spk
                return max(0, 128 * m), m

            def emit_scores(i):
                ph, qc, k = slots[i]
                q0 = qc * TCH
                S = sps.tile([128, 1024], f32, tag="S", name="S")
                if ph == "p1":
                    lo, _ = lo_of(qc, k)
                    nc.tensor.matmul(
                        S[:, lo:TCH],
                        K01[0:64, ts(k, KCH)],
                        Q01[0:64, ds(q0 + lo, TCH - lo)],
                        start=True, stop=True, tile_position=(0, 0))
                    nc.tensor.matmul(
                        S[:, TCH + lo:1024],
                        K01[64:128, ts(k, KCH)],
                        Q01[64:128, ds(q0 + lo, TCH - lo)],
                        start=True, stop=True, tile_position=(64, 0))
                else:
                    kc0, kc1 = 2 * k, 2 * k + 1
                    lo0, _ = lo_of(qc, kc0)
                    nc.tensor.matmul(
                        S[:, lo0:TCH],
                        K2[0:64, ts(kc0, KCH)],
                        Q2[0:64, ds(q0 + lo0, TCH - lo0)],
                        start=True, stop=True, tile_position=(0, 0))
                    # kc1 half also starts at lo0 so the merged strided exp
                    # reads fully-initialized PSUM; [lo0:lo1) is never read.
                    nc.tensor.matmul(
                        S[:, TCH + lo0:1024],
                        K2[64:128, ts(kc1, KCH)],
                        Q2[64:128, ds(q0 + lo0, TCH - lo0)],
                        start=True, stop=True, tile_position=(64, 0))
                s_pend[i] = S

            def slot_slack(i):
                ph, qc, k = slots[i]
                lo, _ = lo_of(qc, k if ph == "p1" else 2 * k)
                w = TCH - lo
                return (2 * w * 0.833 + 270) - (3 * w * 0.4167 + 220)

            def normalize(oT, h, qc):
                den = wkp.tile([1, TCH], f32, tag="den", name="den")
                nc.vector.tensor_copy(den[:], oT[64:65, :])
                recip = wkp.tile([1, TCH], f32, tag="recip", name="recip")
                nc.vector.reciprocal_approx_fast(out=recip[:], in_=den[:])
                rb = wkp.tile([64, TCH], f32, tag="rb", name="rb")
                nc.gpsimd.partition_broadcast(rb[:], recip[:])
                if h == 0:
                    nc.vector.tensor_mul(yTa[0:64, ts(qc, TCH)], oT[0:64, :],
                                         rb[:])
                elif h == 2:
                    nc.vector.tensor_mul(yT2[:, ts(qc, TCH)], oT[0:64, :],
                                         rb[:])
                else:
                    y1t = wkp.tile([64, TCH], bf16, tag="y1t", name="y1t")
                    nc.vector.tensor_mul(y1t[:], oT[0:64, :], rb[:])
                    nc.sync.dma_start(yTa[64:128, ts(qc, TCH)], y1t[:])

            credit = [0.0]

            def drain_due(i):
                """Emit every unit that must precede slot i's emissions."""
                while fq and any(u[0] <= i for u in fq):
                    credit[0] -= fq_pop()

            def pump(i):
                """Deadline-aware pop: spread queued units so every due
                slot is met without bursts; extra pops on spare credit."""
                need = 0
                cnt = 0
                for due, _, _ in fq:
                    cnt += 1
                    if due < ns:
                        rem = max(1, due - i)
                        need = max(need, math.ceil(cnt / rem))
                pops = 0
                while fq and pops < max(need, 3):
                    if pops >= need and (pops >= 2 or credit[0] <= 0):
                        break
                    credit[0] -= fq_pop()
                    pops += 1
                credit[0] += slot_slack(i)
                credit[0] = max(-2000.0, min(3000.0, credit[0]))

            def do_slot(i):
                ph, qc, k = slots[i]
                nkc = (qc + 1) * spk
                npair = nkc // 2
                drain_due(i)
                S = s_pend.pop(i)
                PT = ptp.tile([128, 1024], bf16, tag="PT", name="PT")
                if ph == "p1":
                    lo, m = lo_of(qc, k)
                    if lo == 0:
                        nc.scalar.activation(PT[:], S[:], AF.Exp, scale=0.125)
                    else:
                        s_v = S[:].rearrange("p (h q) -> p h q", h=2)[
                            :, :, lo:TCH]
                        p_v = PT[:].rearrange("p (h q) -> p h q", h=2)[
                            :, :, lo:TCH]
                        nc.scalar.activation(p_v, s_v, AF.Exp, scale=0.125)
                    if m >= 0:
                        nc.vector.tensor_mul(PT[:, ds(lo, 128)],
                                             PT[:, ds(lo, 128)], tri_sb[:])
                        nc.vector.tensor_mul(PT[:, ds(TCH + lo, 128)],
                                             PT[:, ds(TCH + lo, 128)],
                                             tri_sb[:])
                    if k == 0:
                        oT_cur["o0"] = otp.tile([65, TCH], f32, tag="oT",
                                                name="oT0")
                        oT_cur["o1"] = otp.tile([65, TCH], f32, tag="oT",
                                                name="oT1")
                    oT0, oT1 = oT_cur["o0"], oT_cur["o1"]
                    nc.tensor.matmul(oT0[:, lo:TCH], Vp[:, k, 0:65],
                                     PT[:, lo:TCH],
                                     start=(k == 0), stop=(k == nkc - 1))
                    nc.tensor.matmul(oT1[:, lo:TCH], Vp[:, k, 65:130],
                                     PT[:, TCH + lo:1024],
                                     start=(k == 0), stop=(k == nkc - 1))
                else:
                    kc0, kc1 = 2 * k, 2 * k + 1
                    lo0, m0 = lo_of(qc, kc0)
                    lo1, m1 = lo_of(qc, kc1)
                    if lo0 == 0:
                        nc.scalar.activation(PT[:], S[:], AF.Exp, scale=0.125)
                    else:
                        s_v = S[:].rearrange("p (h q) -> p h q", h=2)[
                            :, :, lo0:TCH]
                        p_v = PT[:].rearrange("p (h q) -> p h q", h=2)[
                            :, :, lo0:TCH]
                        nc.scalar.activation(p_v, s_v, AF.Exp, scale=0.125)
                    if m0 >= 0:
                        nc.vector.tensor_mul(PT[:, ds(lo0, 128)],
                                             PT[:, ds(lo0, 128)], tri_sb[:])
                    if m1 >= 0:
                        nc.vector.tensor_mul(PT[:, ds(TCH + lo1, 128)],
                                             PT[:, ds(TCH + lo1, 128)],
                                             tri_sb[:])
                    if k == 0:
                        oT_cur["o2"] = otp.tile([65, TCH], f32, tag="oT",
                                                name="oT2")
                    oT2 = oT_cur["o2"]
                    nc.tensor.matmul(oT2[:, lo0:TCH], Vp[:, kc0, 130:195],
                                     PT[:, lo0:TCH],
                                     start=(k == 0), stop=False)
                    nc.tensor.matmul(oT2[:, lo1:TCH], Vp[:, kc1, 130:195],
                                     PT[:, TCH + lo1:1024],
                                     start=False, stop=(k == npair - 1))
                if i + 2 < ns:
                    emit_scores(i + 2)
                if ph == "p1" and k == nkc - 1:
                    normalize(oT_cur.pop("o0"), 0, qc)
                    normalize(oT_cur.pop("o1"), 1, qc)
                if ph == "p2" and k == npair - 1:
                    normalize(oT_cur.pop("o2"), 2, qc)
                    last = qc == nt - 1
                    for st in range(spk):
                        fq.append(proj_unit(qc, st, tail=last))
                pump(i)

            # ---- preamble.  Startup DMAs issue from three different
            # queues in parallel (each issue costs ~0.6us of queue time):
            # gpsimd ships the k/q weights, sync the x pieces, vector the
            # rest of the packed weights.
            nc.gpsimd.dma_start(wpk[:, 768:1536], wpk_d[:, 768:1536])  # wk01
            xtb0 = xin.tile([128, cc_n, TCH], bf16, tag="xtb", name="xtb")
            xtb_by_tc[0] = xtb0
            nc.sync.dma_start(xtb0[:, 0:3, :], xT_r[:, 0, 0:3, :])
            nc.gpsimd.dma_start(wpk[:, 0:768], wpk_d[:, 0:768])        # wq01
            nc.sync.dma_start(xtb0[:, 3:6, :], xT_r[:, 0, 3:6, :])
            nc.gpsimd.dma_start(bpk[:], bpk_d[:, :])
            nc.scalar.dma_start(wpk[:, 1536:2304], wpk_d[:, 1536:2304])
            nc.scalar.dma_start(wpk[:, 2304:3474], wpk_d[:, 2304:3474])
            nc.scalar.dma_start(wpk[:, 3474:PK_W], wpk_d[:, 3474:PK_W])
            warm_src = consts.tile([128, 512], bf16)
            nc.gpsimd.memset(warm_src[:], 0.0)
            wps = shp.tile([128, 512], f32, tag="sh", name="warm")
            for _ in range(4):
                nc.tensor.matmul(wps[:], warm_src[:, 0:128], warm_src[:],
                                 start=True, stop=True)

            def qk_chain(tci, wsb, bsb, dst, t0, t1):
                qkps = shp.tile([128, t1 - t0], f32, tag="sh", name="qkps")
                xtb = xtb_by_tc[tci]
                for cc in range(cc_n):
                    nc.tensor.matmul(
                        qkps[:], wsb[:, cc, :], xtb[:, cc, t0:t1],
                        start=(cc == 0), stop=(cc == cc_n - 1))
                nc.vector.tensor_scalar_add(
                    dst[:, tci * TCH + t0:tci * TCH + t1], qkps[:], bsb[:])

            qk_chain(0, wk01_sb, bk01_sb, K01, 0, KCH)
            qk_chain(0, wq01_sb, bq01_sb, Q01, 0, TCH)
            emit_scores(0)
            qk_chain(0, wk01_sb, bk01_sb, K01, KCH, TCH)
            emit_scores(1)
            for _, _, fn in qk_units(0, wqk2_sb, bqk2_sb, None, 0):
                fn()
            for st in range(spk):
                for _, _, fn in v_units(0, st):
                    fn()

            # ---- main flat slot loop ----
            for i in range(ns):
                ph, qc, k = slots[i]
                if ph == "p1" and k == 0 and qc + 1 < nt:
                    fq.extend(qkv_all_units(qc + 1))
                do_slot(i)

            # ---- drain remaining fillers (last proj tiles) ----
            while fq:
                fq_pop()

    return nc


def arrange_x(xb):
    """x[b] is [t, C]; device wants xT as [128, nt, cc, TCH] contiguous."""
    import ml_dtypes
    t = xb.shape[0]
    xt = xb.T.reshape(C // CPART, CPART, t // TCH, TCH)
    xt = xt.transpose(1, 2, 0, 3).reshape(CPART, -1)
    return np.ascontiguousarray(xt).astype(ml_dtypes.bfloat16)


def make_tri():
    import ml_dtypes
    p = np.arange(128)[:, None]
    j = np.arange(128)[None, :]
    return (j - p >= 0).astype(ml_dtypes.bfloat16)


def core_inputs(c, x, w_attn, b_attn, w_proj, xT_by_batch, tri):
    import ml_dtypes
    f32 = np.float32
    b = c // 4
    heads = [(c % 4) * HPC + i for i in range(HPC)]
    h0, h1, h2 = heads

    def Wq(h):
        return w_attn[:, h * HD:(h + 1) * HD]

    def Wk(h):
        return w_attn[:, C + h * HD:C + (h + 1) * HD]

    def Wv(h):
        return w_attn[:, 2 * C + h * HD:2 * C + (h + 1) * HD]

    def bq(h):
        return b_attn[h * HD:(h + 1) * HD]

    def bk(h):
        return b_attn[C + h * HD:C + (h + 1) * HD]

    wv195 = np.zeros((C, 195), f32)
    for i, h in enumerate(heads):
        wv195[:, i * 65:i * 65 + 64] = Wv(h)
    bf = ml_dtypes.bfloat16

    def arr(w):
        m = w.shape[1]
        return np.ascontiguousarray(
            w.reshape(C // CPART, CPART, m).transpose(1, 0, 2).reshape(
                CPART, -1)).astype(bf)

    wp192 = np.concatenate([w_proj[h * HD:(h + 1) * HD, :] for h in heads], 0)
    wpB = np.zeros((CPART, C), np.float32)
    wpB[0:64, :] = wp192[128:192, :]
    wpB[64:128, :] = wp192[128:192, :]
    misc = np.zeros((CPART, 384), np.float32)
    wpk = np.concatenate([
        arr(np.concatenate([Wq(h0), Wq(h1)], 1)).astype(np.float32),
        arr(np.concatenate([Wk(h0), Wk(h1)], 1)).astype(np.float32),
        arr(np.concatenate([Wq(h2), Wk(h2)], 1)).astype(np.float32),
        arr(wv195).astype(np.float32),
        wp192[0:128, :], wpB, tri.astype(np.float32), misc,
    ], axis=1).astype(bf)
    bpk = np.stack([
        np.concatenate([bq(h0), bq(h1)]),
        np.concatenate([bk(h0), bk(h1)]),
        np.concatenate([bq(h2), bk(h2)]),
    ], axis=1).astype(np.float32)
    return {
        "xT": xT_by_batch[b],
        "wpk": np.ascontiguousarray(wpk),
        "bpk": np.ascontiguousarray(bpk),
    }


TRACE = False
LAST_EXEC_NS = None
LAST_RESULTS = None


def kernel(x, w_attn, b_attn, w_proj, b_proj):
    global LAST_EXEC_NS, LAST_RESULTS
    _ensure_axon_hooks_module()
    from concourse.bass_utils import run_bass_kernel_spmd

    x = np.asarray(x, np.float32)
    w_attn = np.asarray(w_attn, np.float32)
    b_attn = np.asarray(b_attn, np.float32)
    w_proj = np.asarray(w_proj, np.float32)
    b_proj = np.asarray(b_proj, np.float32)

    if "nc" not in _cache:
        nc = build_program()
        nc.finalize()
        _cache["nc"] = nc
    nc = _cache["nc"]

    import ml_dtypes  # noqa: F401
    xT_by_batch = [arrange_x(x[b]) for b in range(B)]
    tri = make_tri()
    in_maps = [
        core_inputs(c, x, w_attn, b_attn, w_proj, xT_by_batch, tri)
        for c in range(N_CORES)
    ]
    res = run_bass_kernel_spmd(nc, in_maps, core_ids=list(range(N_CORES)),
                               trace=TRACE)
    LAST_EXEC_NS = res.exec_time_ns
    LAST_RESULTS = res
    out = np.zeros((B, T, C), np.float32)
    for c in range(N_CORES):
        out[c // 4] += np.asarray(res.results[c]["out"], np.float32)
    # V-bias commutes out of softmax (weights sum to 1): add bv @ w_proj
    bv_all = b_attn[2 * C:]
    out += (b_proj + bv_all @ w_proj)[None, None, :]
    return out


# revision 15
# speedup vs baseline: 1.0856x; 1.0605x over previous
"""Causal self-attention (GPT-style, B=2, T=4096, C=768, 12 heads) on 8 TRN2
NeuronCores.

Sharding: core c handles batch b = c//4 and heads [3g, 3g+1, 3g+2] with
g = c%4 (data parallel on B x tensor parallel on heads).  Each core computes
its heads' attention output projected through its slice of w_proj; the host
sums the 4 partial [T, C] outputs per batch and adds b_proj.

Device-side formulation (all matmuls bf16, fp32 accumulate):
  - host passes x[b].T so QKV projections contract C on partitions:
      qT/kT  = W.T @ x.T        -> [head_dim(=partitions), T]
      V'     = x @ [Wv|0] + ones-col -> [T(=partitions), 3*65]  (col 64 of
               each 65-block is constant 1 -> PV also yields softmax denoms)
  - scores computed transposed, S^T[k, q] via lhsT=kT, rhs=qT; two heads per
    512-cycle slot via PE row-tiling (K=64 each, concurrent row groups).
  - softmax without max-subtraction (scores are O(5), exp safe in fp32):
      P^T = exp(0.125 * S^T) on ScalarE, PSUM->SBUF bf16, one activate per
    [128, 1024] (both heads / both k-chunks of a slot share it).
  - causal: strictly-masked k-chunks never computed; at boundary tiles the
    score matmul / exp / PV shrink to the valid column subrange and only the
    diagonal [128,128] strip is multiplied by a triangular 0/1 bf16 mask.
  - PV: oT'[65, q] += V'[k,65].T @ P^T[k,q] accumulated over k-chunks; row 64
    is the softmax denominator.  Normalize: approx-reciprocal on DVE, GpSimd
    partition_broadcast across the 64 head-dim partitions, one DVE multiply.
  - output projection (packed K=128): out[t,:] = yTa[:,t].T @ wp[h01-rows] +
    yT2[:,t].T @ wp[h2-rows].

Schedule (the key difference vs the naive per-phase loop): the ScalarE exp
stream is the critical resource (~1.03us per slot, 216 slots/core), so the
whole kernel is driven as ONE flat sequence of attention slots with the
score matmuls emitted two slots ahead (double-buffered PSUM) ACROSS phase
and q-chunk boundaries, keeping the exp cadence unbroken.  All other PE work
(QKV projection, V' projection, output projection) is chopped into ~0.4-0.8us
filler units in a FIFO queue, popped 1-3 per slot under a slack-credit model
so TensorE stays fed without starving the exp stream.  PSUM banks: 4 for the
two in-flight score tiles, 2 for the PV accumulators, 2 shared by the
qkv/proj/V' accumulation chains.
"""

import numpy as np

N_CORES = 8
B = 2
T = 4096
C = 768
NH = 12
HD = 64
HPC = 3            # heads per core
TCH = 512          # t / q chunk
KCH = 128          # k chunk
CPART = 128

_cache = {}


def _ensure_axon_hooks_module():
    """Make `from antenv.axon_hooks import ...` importable even on images
    whose antenv package lacks the module (profiling then degrades to a
    no-op instead of crashing run_bass_kernel_spmd(trace=True))."""
    import sys
    import types
    try:
        import antenv.axon_hooks  # noqa: F401
        return
    except Exception:
        pass
    m = types.ModuleType("antenv.axon_hooks")
    m._hook = None

    def _set(h):
        m._hook = h

    def _get():
        return m._hook

    m.set_axon_ntff_profile_hook = _set
    m.get_axon_ntff_profile_hook = _get
    sys.modules["antenv.axon_hooks"] = m


def build_program(t=T):
    """Build the single-core SPMD bass program (same program on all cores,
    per-core data). Returns the un-finalized Bacc."""
    import math
    from collections import deque

    import concourse.mybir as mybir
    import concourse.tile as tile
    from concourse import bacc
    from concourse.bass import ds, ts

    f32 = mybir.dt.float32
    bf16 = mybir.dt.bfloat16
    AF = mybir.ActivationFunctionType

    nt = t // TCH          # number of t/q chunks
    spk = TCH // KCH       # k-chunks per t-chunk (4)
    cc_n = C // CPART      # 6 contraction chunks

    nc = bacc.Bacc("TRN2", target_bir_lowering=False)

    # packed bf16 constants: [wq01 768 | wk01 768 | wqk2 768 | wv 1152 |
    #  wpA 768 | wpB 768 (rows 0:64) | tri 128 | misc 384]
    PK_W = 6 * 128 * 3 + 6 * 192 + C + C + 128 + 384
    xT = nc.dram_tensor("xT", [128, (t // TCH) * (C // CPART) * TCH], bf16,
                        kind="ExternalInput")
    wpk_d = nc.dram_tensor("wpk", [128, PK_W], bf16, kind="ExternalInput")
    bpk_d = nc.dram_tensor("bpk", [128, 3], f32, kind="ExternalInput")
    out_d = nc.dram_tensor("out", [t, C], bf16, kind="ExternalOutput")

    with tile.TileContext(nc) as tc_:
        with (
            tc_.tile_pool(name="consts", bufs=1) as consts,
            tc_.tile_pool(name="big", bufs=1) as big,
            tc_.tile_pool(name="xin", bufs=3) as xin,
            tc_.tile_pool(name="ptp", bufs=8) as ptp,
            tc_.tile_pool(name="wkp", bufs=4) as wkp,
            tc_.tile_pool(name="sps", bufs=2, space="PSUM") as sps,
            tc_.tile_pool(name="otp", bufs=2, space="PSUM") as otp,
            tc_.tile_pool(name="shp", bufs=2, space="PSUM") as shp,
        ):
            wpk = consts.tile([128, PK_W], bf16)
            bpk = consts.tile([128, 3], f32)

            def seg(off, w):
                ap = wpk[:, off:off + w]
                return ap, off + w

            _o = 0
            wq01_f, _o = seg(_o, 6 * 128)
            wk01_f, _o = seg(_o, 6 * 128)
            wqk2_f, _o = seg(_o, 6 * 128)
            wv_f, _o = seg(_o, 6 * 192)
            wpA_sb, _o = seg(_o, C)
            wpB_full, _o = seg(_o, C)
            tri_sb, _o = seg(_o, 128)
            misc_f, _o = seg(_o, 384)
            wq01_sb = wq01_f.rearrange("p (c m) -> p c m", c=cc_n)
            wk01_sb = wk01_f.rearrange("p (c m) -> p c m", c=cc_n)
            wqk2_sb = wqk2_f.rearrange("p (c m) -> p c m", c=cc_n)
            wv_sb = wv_f.rearrange("p (c m) -> p c m", c=cc_n)
            wpB_lo = wpB_full[0:64, :]
            del misc_f
            bq01_sb = bpk[:, 0:1]
            bk01_sb = bpk[:, 1:2]
            bqk2_sb = bpk[:, 2:3]

            # ---- persistent activations ----
            Q01 = big.tile([128, t], bf16)   # rows 0-63 qT_h0, 64-127 qT_h1
            K01 = big.tile([128, t], bf16)
            Q2 = big.tile([128, t], bf16)    # qT_h2 duplicated on both halves
            K2 = big.tile([128, t], bf16)
            Vp = big.tile([128, t // KCH, 384], bf16)
            Vp_h = Vp[:, :, :].rearrange("p n (h x) -> p n h x", h=3)
            yTa = big.tile([128, t], bf16)   # normalized h0 (0:64) | h1
            yT2 = big.tile([64, t], bf16)    # h2

            xT_r = xT[:, :].rearrange("p (nt c m) -> p nt c m", nt=nt,
                                      c=cc_n)

            # ---- slot table ----
            slots = []
            qc_start = []
            for qc in range(nt):
                nkc = (qc + 1) * spk
                qc_start.append(len(slots))
                slots += [("p1", qc, kc) for kc in range(nkc)]
                slots += [("p2", qc, kp) for kp in range(nkc // 2)]
            ns = len(slots)

            def p2_start(qc):
                return qc_start[qc] + (qc + 1) * spk

            # ---- filler units: FIFO of (due_slot, cost, fn) ----
            # Emission order IS dependency order on the in-order engine
            # queues, so every unit carries a due slot: the latest slot
            # index by which it must have been EMITTED.  The pump spreads
            # pops so each deadline is met without bursts.
            fq = deque()
            xtb_by_tc = {}

            def fq_pop():
                due, cost, fn = fq.popleft()
                fn()
                return cost

            def unit_xdma(tci):
                def fn():
                    xtb = xin.tile([128, cc_n, TCH], bf16, tag="xtb",
                                   name="xtb")
                    nc.gpsimd.dma_start(xtb[:], xT_r[:, tci, :, :])
                    xtb_by_tc[tci] = xtb
                return (max(0, qc_start[tci] - 8), 30, fn)

            def qk_units(tci, wsb, bsb, dst, due):
                st8 = {}

                def mk(cc0):
                    def fn():
                        if cc0 == 0:
                            st8["ps"] = shp.tile([128, TCH], f32, tag="sh",
                                                 name="qkps")
                        qkps = st8["ps"]
                        xtb = xtb_by_tc[tci]
                        for cc in (cc0, cc0 + 1):
                            nc.tensor.matmul(
                                qkps[:], wsb[:, cc, :], xtb[:, cc, :],
                                start=(cc == 0), stop=(cc == cc_n - 1))
                    return fn

                def bias_fn():
                    qkps = st8.pop("ps")
                    if dst is None:
                        nc.vector.tensor_scalar_add(
                            Q2[0:64, ts(tci, TCH)], qkps[0:64, :],
                            bsb[0:64, :])
                        nc.vector.tensor_scalar_add(
                            K2[64:128, ts(tci, TCH)], qkps[64:128, :],
                            bsb[64:128, :])
                        nc.sync.dma_start(Q2[64:128, ts(tci, TCH)],
                                          Q2[0:64, ts(tci, TCH)])
                        nc.sync.dma_start(K2[0:64, ts(tci, TCH)],
                                          K2[64:128, ts(tci, TCH)])
                    else:
                        nc.vector.tensor_scalar_add(
                            dst[:, ts(tci, TCH)], qkps[:], bsb[:])

                return ([(due, 490, mk(cc0)) for cc0 in (0, 2, 4)]
                        + [(due, 60, bias_fn)])

            def v_units(tci, st):
                """V' projection for one 128-token slice; due right before
                the diagonal PV slot that reads it."""
                tt = tci * spk + st
                due = qc_start[tci] + tci * spk + st - 1 if tci > 0 else 0
                st8 = {}

                def fn_a():
                    st8["ps"] = shp.tile([128, 192], f32, tag="sh",
                                         name="vps")
                    xtb = xtb_by_tc[tci]
                    for cc in (0, 1, 2):
                        nc.tensor.matmul(
                            st8["ps"][:], xtb[:, cc, ts(st, 128)],
                            wv_sb[:, cc, :],
                            start=(cc == 0), stop=False)

                def fn_b():
                    vps = st8.pop("ps")
                    xtb = xtb_by_tc[tci]
                    for cc in (3, 4, 5):
                        nc.tensor.matmul(
                            vps[:], xtb[:, cc, ts(st, 128)], wv_sb[:, cc, :],
                            start=False, stop=(cc == cc_n - 1))
                    nc.vector.tensor_copy(
                        Vp_h[:, tt, :, 0:64],
                        vps[:].rearrange("p (h x) -> p h x", h=3))
                    nc.vector.memset(Vp_h[:, tt, :, 64:128], 1.0)

                return [(due - 1, 420, fn_a), (due, 480, fn_b)]

            def qkv_all_units(tci):
                u = [unit_xdma(tci)]
                due_qk = qc_start[tci] - 2
                u += qk_units(tci, wq01_sb, bq01_sb, Q01, due_qk)
                u += qk_units(tci, wk01_sb, bk01_sb, K01, due_qk)
                u += qk_units(tci, wqk2_sb, bqk2_sb, None,
                              p2_start(tci) - 2)
                for st in range(spk):
                    u += v_units(tci, st)
                return u

            def proj_unit(tci, st, tail=False):
                tt = tci * spk + st
                due = qc_start[tci + 2] if tci + 2 < nt else ns - 1

                def fn():
                    po1 = shp.tile([128, 512], f32, tag="sh", name="po1")
                    po2 = shp.tile([128, 256], f32, tag="sh", name="po2")
                    for po, cs, cw in ((po1, 0, 512), (po2, 512, 256)):
                        nc.tensor.matmul(po[:], yTa[:, ts(tt, 128)],
                                         wpA_sb[:, ds(cs, cw)],
                                         start=True, stop=False)
                        nc.tensor.matmul(po[:], yT2[:, ts(tt, 128)],
                                         wpB_lo[:, ds(cs, cw)],
                                         start=False, stop=True)
                    pout = xin.tile([128, C], bf16, tag="pout", name="pout")
                    if tail:
                        nc.scalar.activation(pout[:, 0:512], po1[:], AF.Copy)
                    else:
                        nc.vector.tensor_copy(pout[:, 0:512], po1[:])
                    nc.vector.tensor_copy(pout[:, 512:768], po2[:])
                    nc.sync.dma_start(out_d[ds(tt * 128, 64), :],
                                      pout[0:64, :])
                    nc.sync.dma_start(out_d[ds(tt * 128 + 64, 64), :],
                                      pout[64:128, :])
                return (due, 760, fn)

            # ---- attention machinery ----
            s_pend = {}
            oT_cur = {}

            def lo_of(qc, kc):
                m = kc - qc * spk
                return max(0, 128 * m), m

            def emit_scores(i):
                ph, qc, k = slots[i]
                q0 = qc * TCH
                S = sps.tile([128, 1024], f32, tag="S", name="S")
                if ph == "p1":
                    lo, _ = lo_of(qc, k)
                    nc.tensor.matmul(
                        S[:, lo:TCH],
                        K01[0:64, ts(k, KCH)],
                        Q01[0:64, ds(q0 + lo, TCH - lo)],
                        start=True, stop=True, tile_position=(0, 0))
                    nc.tensor.matmul(
                        S[:, TCH + lo:1024],
                        K01[64:128, ts(k, KCH)],
                        Q01[64:128, ds(q0 + lo, TCH - lo)],
                        start=True, stop=True, tile_position=(64, 0))
                else:
                    kc0, kc1 = 2 * k, 2 * k + 1
                    lo0, _ = lo_of(qc, kc0)
                    nc.tensor.matmul(
                        S[:, lo0:TCH],
                        K2[0:64, ts(kc0, KCH)],
                        Q2[0:64, ds(q0 + lo0, TCH - lo0)],
                        start=True, stop=True, tile_position=(0, 0))
                    # kc1 half also starts at lo0 so the merged strided exp
                    # reads fully-initialized PSUM; [lo0:lo1) is never read.
                    nc.tensor.matmul(
                        S[:, TCH + lo0:1024],
                        K2[64:128, ts(kc1, KCH)],
                        Q2[64:128, ds(q0 + lo0, TCH - lo0)],
                        start=True, stop=True, tile_position=(64, 0))
                s_pend[i] = S

            def slot_slack(i):
                ph, qc, k = slots[i]
                lo, _ = lo_of(qc, k if ph == "p1" else 2 * k)
                w = TCH - lo
                return (2 * w * 0.833 + 270) - (3 * w * 0.4167 + 220)

            def normalize(oT, h, qc):
                den = wkp.tile([64, TCH], f32, tag="den", name="den")
                nc.vector.tensor_copy(den[:], oT[64:128, :])
                rb = wkp.tile([64, TCH], f32, tag="rb", name="rb")
                nc.vector.reciprocal_approx_fast(out=rb[:], in_=den[:])
                if h == 0:
                    nc.vector.tensor_mul(yTa[0:64, ts(qc, TCH)], oT[0:64, :],
                                         rb[:])
                elif h == 2:
                    nc.vector.tensor_mul(yT2[:, ts(qc, TCH)], oT[0:64, :],
                                         rb[:])
                else:
                    y1t = wkp.tile([64, TCH], bf16, tag="y1t", name="y1t")
                    nc.vector.tensor_mul(y1t[:], oT[0:64, :], rb[:])
                    nc.sync.dma_start(yTa[64:128, ts(qc, TCH)], y1t[:])

            credit = [0.0]

            def drain_due(i):
                """Emit every unit that must precede slot i's emissions."""
                while fq and any(u[0] <= i for u in fq):
                    credit[0] -= fq_pop()

            def pump(i):
                """Deadline-aware pop: spread queued units so every due
                slot is met without bursts; extra pops on spare credit."""
                need = 0
                cnt = 0
                for due, _, _ in fq:
                    cnt += 1
                    if due < ns:
                        rem = max(1, due - i)
                        need = max(need, math.ceil(cnt / rem))
                pops = 0
                while fq and pops < max(need, 3):
                    if pops >= need and (pops >= 2 or credit[0] <= 0):
                        break
                    credit[0] -= fq_pop()
                    pops += 1
                credit[0] += slot_slack(i)
                credit[0] = max(-2000.0, min(3000.0, credit[0]))

            def do_slot(i):
                ph, qc, k = slots[i]
                nkc = (qc + 1) * spk
                npair = nkc // 2
                drain_due(i)
                S = s_pend.pop(i)
                PT = ptp.tile([128, 1024], bf16, tag="PT", name="PT")
                if ph == "p1":
                    lo, m = lo_of(qc, k)
                    if lo == 0:
                        nc.scalar.activation(PT[:], S[:], AF.Exp, scale=0.125)
                    else:
                        s_v = S[:].rearrange("p (h q) -> p h q", h=2)[
                            :, :, lo:TCH]
                        p_v = PT[:].rearrange("p (h q) -> p h q", h=2)[
                            :, :, lo:TCH]
                        nc.scalar.activation(p_v, s_v, AF.Exp, scale=0.125)
                    if m >= 0:
                        nc.vector.tensor_mul(PT[:, ds(lo, 128)],
                                             PT[:, ds(lo, 128)], tri_sb[:])
                        nc.vector.tensor_mul(PT[:, ds(TCH + lo, 128)],
                                             PT[:, ds(TCH + lo, 128)],
                                             tri_sb[:])
                    if k == 0:
                        oT_cur["o0"] = otp.tile([128, TCH], f32, tag="oT",
                                                name="oT0")
                        oT_cur["o1"] = otp.tile([128, TCH], f32, tag="oT",
                                                name="oT1")
                    oT0, oT1 = oT_cur["o0"], oT_cur["o1"]
                    nc.tensor.matmul(oT0[:, lo:TCH], Vp[:, k, 0:128],
                                     PT[:, lo:TCH],
                                     start=(k == 0), stop=(k == nkc - 1))
                    nc.tensor.matmul(oT1[:, lo:TCH], Vp[:, k, 128:256],
                                     PT[:, TCH + lo:1024],
                                     start=(k == 0), stop=(k == nkc - 1))
                else:
                    kc0, kc1 = 2 * k, 2 * k + 1
                    lo0, m0 = lo_of(qc, kc0)
                    lo1, m1 = lo_of(qc, kc1)
                    if lo0 == 0:
                        nc.scalar.activation(PT[:], S[:], AF.Exp, scale=0.125)
                    else:
                        s_v = S[:].rearrange("p (h q) -> p h q", h=2)[
                            :, :, lo0:TCH]
                        p_v = PT[:].rearrange("p (h q) -> p h q", h=2)[
                            :, :, lo0:TCH]
                        nc.scalar.activation(p_v, s_v, AF.Exp, scale=0.125)
                    if m0 >= 0:
                        nc.vector.tensor_mul(PT[:, ds(lo0, 128)],
                                             PT[:, ds(lo0, 128)], tri_sb[:])
                    if m1 >= 0:
                        nc.vector.tensor_mul(PT[:, ds(TCH + lo1, 128)],
                                             PT[:, ds(TCH + lo1, 128)],
                                             tri_sb[:])
                    if k == 0:
                        oT_cur["o2"] = otp.tile([128, TCH], f32, tag="oT",
                                                name="oT2")
                    oT2 = oT_cur["o2"]
                    nc.tensor.matmul(oT2[:, lo0:TCH], Vp[:, kc0, 256:384],
                                     PT[:, lo0:TCH],
                                     start=(k == 0), stop=False)
                    nc.tensor.matmul(oT2[:, lo1:TCH], Vp[:, kc1, 256:384],
                                     PT[:, TCH + lo1:1024],
                                     start=False, stop=(k == npair - 1))
                if i + 2 < ns:
                    emit_scores(i + 2)
                if ph == "p1" and k == nkc - 1:
                    normalize(oT_cur.pop("o0"), 0, qc)
                    normalize(oT_cur.pop("o1"), 1, qc)
                if ph == "p2" and k == npair - 1:
                    normalize(oT_cur.pop("o2"), 2, qc)
                    last = qc == nt - 1
                    for st in range(spk):
                        fq.append(proj_unit(qc, st, tail=last))
                pump(i)

            # ---- preamble.  Startup DMAs issue from three different
            # queues in parallel (each issue costs ~0.6us of queue time):
            # gpsimd ships the k/q weights, sync the x pieces, vector the
            # rest of the packed weights.
            warm_src = consts.tile([128, 512], bf16)
            nc.vector.memset(warm_src[:], 0.0)
            nc.gpsimd.dma_start(wpk[:, 768:1536], wpk_d[:, 768:1536])  # wk01
            xtb0 = xin.tile([128, cc_n, TCH], bf16, tag="xtb", name="xtb")
            xtb_by_tc[0] = xtb0
            nc.sync.dma_start(xtb0[:, 0:3, :], xT_r[:, 0, 0:3, :])
            nc.gpsimd.dma_start(wpk[:, 0:768], wpk_d[:, 0:768])        # wq01
            nc.sync.dma_start(xtb0[:, 3:6, :], xT_r[:, 0, 3:6, :])
            nc.gpsimd.dma_start(bpk[:], bpk_d[:, :])
            nc.scalar.dma_start(wpk[:, 1536:2304], wpk_d[:, 1536:2304])
            nc.scalar.dma_start(wpk[:, 2304:3474], wpk_d[:, 2304:3474])
            nc.scalar.dma_start(wpk[:, 3474:PK_W], wpk_d[:, 3474:PK_W])
            wps = sps.tile([128, 1024], f32, tag="S", name="warm")
            for _ in range(4):
                nc.tensor.matmul(wps[:, 0:512], warm_src[:, 0:128],
                                 warm_src[:], start=True, stop=True)

            def qk_chain(tci, wsb, bsb, dst, t0, t1):
                qkps = shp.tile([128, t1 - t0], f32, tag="sh", name="qkps")
                xtb = xtb_by_tc[tci]
                for cc in range(cc_n):
                    nc.tensor.matmul(
                        qkps[:], wsb[:, cc, :], xtb[:, cc, t0:t1],
                        start=(cc == 0), stop=(cc == cc_n - 1))
                nc.vector.tensor_scalar_add(
                    dst[:, tci * TCH + t0:tci * TCH + t1], qkps[:], bsb[:])

            qk_chain(0, wk01_sb, bk01_sb, K01, 0, KCH)
            qk_chain(0, wq01_sb, bq01_sb, Q01, 0, TCH)
            emit_scores(0)
            qk_chain(0, wk01_sb, bk01_sb, K01, KCH, TCH)
            emit_scores(1)
            for _, _, fn in qk_units(0, wqk2_sb, bqk2_sb, None, 0):
                fn()
            for st in range(spk):
                for _, _, fn in v_units(0, st):
                    fn()

            # ---- main flat slot loop ----
            for i in range(ns):
                ph, qc, k = slots[i]
                if ph == "p1" and k == 0 and qc + 1 < nt:
                    fq.extend(qkv_all_units(qc + 1))
                do_slot(i)

            # keep the PE p-state hot through the final normalize
            wps2 = sps.tile([128, 1024], f32, tag="S", name="warm2")
            for _ in range(3):
                nc.tensor.matmul(wps2[:, 0:512], warm_src[:, 0:128],
                                 warm_src[:], start=True, stop=True)
            # ---- drain remaining fillers (last proj tiles) ----
            while fq:
                fq_pop()

    return nc


def arrange_x(xb):
    """x[b] is [t, C]; device wants xT as [128, nt, cc, TCH] contiguous."""
    import ml_dtypes
    t = xb.shape[0]
    xt = xb.T.reshape(C // CPART, CPART, t // TCH, TCH)
    xt = xt.transpose(1, 2, 0, 3).reshape(CPART, -1)
    return np.ascontiguousarray(xt).astype(ml_dtypes.bfloat16)


def make_tri():
    import ml_dtypes
    p = np.arange(128)[:, None]
    j = np.arange(128)[None, :]
    return (j - p >= 0).astype(ml_dtypes.bfloat16)


def core_inputs(c, x, w_attn, b_attn, w_proj, xT_by_batch, tri):
    import ml_dtypes
    f32 = np.float32
    b = c // 4
    heads = [(c % 4) * HPC + i for i in range(HPC)]
    h0, h1, h2 = heads

    def Wq(h):
        return w_attn[:, h * HD:(h + 1) * HD]

    def Wk(h):
        return w_attn[:, C + h * HD:C + (h + 1) * HD]

    def Wv(h):
        return w_attn[:, 2 * C + h * HD:2 * C + (h + 1) * HD]

    def bq(h):
        return b_attn[h * HD:(h + 1) * HD]

    def bk(h):
        return b_attn[C + h * HD:C + (h + 1) * HD]

    wv192 = np.zeros((C, 192), f32)
    for i, h in enumerate(heads):
        wv192[:, i * 64:(i + 1) * 64] = Wv(h)
    bf = ml_dtypes.bfloat16

    def arr(w):
        m = w.shape[1]
        return np.ascontiguousarray(
            w.reshape(C // CPART, CPART, m).transpose(1, 0, 2).reshape(
                CPART, -1)).astype(bf)

    wp192 = np.concatenate([w_proj[h * HD:(h + 1) * HD, :] for h in heads], 0)
    wpB = np.zeros((CPART, C), np.float32)
    wpB[0:64, :] = wp192[128:192, :]
    wpB[64:128, :] = wp192[128:192, :]
    misc = np.zeros((CPART, 384), np.float32)
    wpk = np.concatenate([
        arr(np.concatenate([Wq(h0), Wq(h1)], 1)).astype(np.float32),
        arr(np.concatenate([Wk(h0), Wk(h1)], 1)).astype(np.float32),
        arr(np.concatenate([Wq(h2), Wk(h2)], 1)).astype(np.float32),
        arr(wv192).astype(np.float32),
        wp192[0:128, :], wpB, tri.astype(np.float32), misc,
    ], axis=1).astype(bf)
    bpk = np.stack([
        np.concatenate([bq(h0), bq(h1)]),
        np.concatenate([bk(h0), bk(h1)]),
        np.concatenate([bq(h2), bk(h2)]),
    ], axis=1).astype(np.float32)
    return {
        "xT": xT_by_batch[b],
        "wpk": np.ascontiguousarray(wpk),
        "bpk": np.ascontiguousarray(bpk),
    }


TRACE = False
LAST_EXEC_NS = None
LAST_RESULTS = None


def kernel(x, w_attn, b_attn, w_proj, b_proj):
    global LAST_EXEC_NS, LAST_RESULTS
    _ensure_axon_hooks_module()
    from concourse.bass_utils import run_bass_kernel_spmd

    x = np.asarray(x, np.float32)
    w_attn = np.asarray(w_attn, np.float32)
    b_attn = np.asarray(b_attn, np.float32)
    w_proj = np.asarray(w_proj, np.float32)
    b_proj = np.asarray(b_proj, np.float32)

    if "nc" not in _cache:
        nc = build_program()
        nc.finalize()
        _cache["nc"] = nc
    nc = _cache["nc"]

    import ml_dtypes  # noqa: F401
    xT_by_batch = [arrange_x(x[b]) for b in range(B)]
    tri = make_tri()
    in_maps = [
        core_inputs(c, x, w_attn, b_attn, w_proj, xT_by_batch, tri)
        for c in range(N_CORES)
    ]
    res = run_bass_kernel_spmd(nc, in_maps, core_ids=list(range(N_CORES)),
                               trace=TRACE)
    LAST_EXEC_NS = res.exec_time_ns
    LAST_RESULTS = res
    out = np.zeros((B, T, C), np.float32)
    for c in range(N_CORES):
        out[c // 4] += np.asarray(res.results[c]["out"], np.float32)
    # V-bias commutes out of softmax (weights sum to 1): add bv @ w_proj
    bv_all = b_attn[2 * C:]
    out += (b_proj + bv_all @ w_proj)[None, None, :]
    return out
